# revision 49
# baseline (speedup 1.0000x reference)
"""MetapathAttentionLayer Trainium2 kernel.

Math (per node n):
    scores[n, m] = sum_d x[m, n, d] * W[d, m]
    att = softmax(relu(scores), axis=m)      (8 metapaths)
    out[n, :] = elu(sum_m att[n, m] * x[m, n, :])

Strategy: shard nodes across 8 cores (data parallel). Per core, n-major
layout [nodes(part), d(free)] in bf16, 14 tiles of 7x128-node chunks,
3-stage software pipeline front(t) / pool(t-1) / elu(t-2):
  - one batched DMA per tile each way (in: all metapaths; out: bf16 of
    out+1, host subtracts the 1)
  - scores: DVE 2x-mode product vs replicated-W + one 2x bf16 fold
    (d 128->64), then 64 PE matmuls against an identity rhs accumulate
    the transposed scores [56=(m,c), 128=n] in f32 PSUM (reduce via
    transpose; Ldweights is free, matmul cost = out columns)
  - softmax in transposed layout on ACT (relu+exp), transposed back to
    n-major via a matmul against a host-uploaded 56x56 permutation;
    sums/recip/att on DVE
  - pooling: PE matmuls with diag(att_m) stationary accumulating over m
    in PSUM; diag built per chunk on Pool (local_scatter) / ACT
    (copy-with-scale-ptr) / DVE (tensor_scalar) per diag_sched to
    balance engine load
  - elu(x)+1 = relu(x) + exp(-relu(-x)) on ACT (bf16) + one 2x DVE add
"""

import os
from contextlib import ExitStack

import numpy as np
import ml_dtypes

import concourse.bass as bass
import concourse.tile as tile
from concourse import bacc, mybir, library_config
import concourse.bass_utils as bass_utils

F32 = mybir.dt.float32
BF16 = mybir.dt.bfloat16
I16 = mybir.dt.int16
ALU = mybir.AluOpType
ACTF = mybir.ActivationFunctionType

NMETA = 8
N = 100000
D = 128
NCORES = 8
NC_RAW = N // NCORES          # 12500 nodes per core
CHUNK = 128                   # nodes per compute chunk (partition dim)
NC_PAD = 12544                # 98 chunks of 128
CT = 7                        # chunks per DMA tile
NT = CT * CHUNK               # 896 nodes per tile
NTILES = NC_PAD // NT         # 14
GROUPS = ((0, 4), (4, 3))     # chunk groups per tile (psum bank = 512 f32)

# of every DIAG_MOD chunks, the first DIAG_POOL go to Pool local_scatter,
# the rest to ACT copy-scale (engine balance)
DIAG_MOD = 7
DIAG_POOL = 8
FOLD_TO = 2                   # fold d down to this width in bf16, then f32 reduce


def kernel_body(tc, out_d, x_d, wb_d, sidx_d, icat_d, perm_d, reps=1,
                diag_mod=DIAG_MOD, diag_pool=DIAG_POOL, fold_to=FOLD_TO,
                lag=1, xbufs=4, order="fpe",
                act_pos=(2, 5), pe_reduce=True,
                diag_sched="PPHPPHPHPHPHPP", dcbufs=6, ebufs=4,
                hsplit=4):
    nc = tc.nc
    with ExitStack() as ctx:
        const = ctx.enter_context(tc.tile_pool(name="const", bufs=1))
        xpool = ctx.enter_context(tc.tile_pool(name="x", bufs=xbufs))
        opool = ctx.enter_context(tc.tile_pool(name="o", bufs=3))
        ppool = ctx.enter_context(tc.tile_pool(name="prod", bufs=3))
        f1pool = ctx.enter_context(tc.tile_pool(name="f1", bufs=3))
        f2pool = ctx.enter_context(tc.tile_pool(name="f2", bufs=3))
        spool = ctx.enter_context(tc.tile_pool(name="smalls", bufs=8))
        apool = ctx.enter_context(tc.tile_pool(name="att", bufs=4))
        dcpool = ctx.enter_context(tc.tile_pool(name="diagC", bufs=dcbufs))
        epool = ctx.enter_context(tc.tile_pool(name="elu", bufs=ebufs))
        psum = ctx.enter_context(tc.tile_pool(name="ps", bufs=4, space="PSUM"))
        psum2 = ctx.enter_context(tc.tile_pool(name="ps2", bufs=2,
                                               space="PSUM"))

        wb = const.tile([128, NMETA * D], BF16)
        nc.sync.dma_start(wb[:], wb_d[:])
        sidx = const.tile([128, NMETA], I16)
        nc.sync.dma_start(sidx[:], sidx_d[:])
        icat = const.tile([128, NMETA * D], BF16)
        nc.sync.dma_start(icat[:], icat_d[:])
        permt = const.tile([128, NMETA * CT + CT], BF16)
        nc.sync.dma_start(permt[:NMETA * CT, :], perm_d[:])
        nc.gpsimd.load_library(library_config.local_scatter)

        wbv = wb[:].rearrange("p (m d) -> p m d", m=NMETA)

        for _rep in range(reps):
            # software pipeline over tiles: front = DMA + scores + softmax,
            # back (lag tiles later) = diag + pooling + elu + store.
            tiles = {}

            def emit_front(t):
                n0 = t * NT
                X = xpool.tile([128, NMETA * NT], BF16, tag="X")
                src = x_d[:, n0:n0 + NT, :].rearrange(
                    "m (p c) d -> p m c d", p=128)
                Xv = X[:].rearrange("p (m c d) -> p m c d", m=NMETA, c=CT)
                nc.sync.dma_start(Xv, src)
                outp = opool.tile([128, NT], BF16, tag="outp")
                tiles[t] = {"Xv": Xv, "outp": outp}

                if pe_reduce:
                    # product + one bf16 fold level; then PE reduces the
                    # remaining 64 d-slices into PSUM (f32 accumulate),
                    # yielding transposed scores [56=(m c), 128=n].
                    F1t = f1pool.tile([128, NMETA * CT * 64], BF16,
                                      tag="F1t")
                    F1v = F1t[:].rearrange("p (m c d) -> p m c d",
                                           m=NMETA, c=CT)
                    for g0, gl in GROUPS:
                        P = ppool.tile([128, NMETA * 4 * D], BF16, tag="P")
                        Pv = P[:].rearrange("p (m g d) -> p m g d",
                                            m=NMETA, g=4)
                        nc.vector.tensor_tensor(
                            out=Pv[:, :, :gl, :],
                            in0=Xv[:, :, g0:g0 + gl, :],
                            in1=wbv.unsqueeze(2).broadcast_to(
                                [128, NMETA, gl, D]),
                            op=ALU.mult,
                        )
                        with nc.allow_low_precision(reason="bf16 fold"):
                            nc.vector.tensor_tensor(
                                out=F1v[:, :, g0:g0 + gl, :],
                                in0=Pv[:, :, :gl, :64],
                                in1=Pv[:, :, :gl, 64:],
                                op=ALU.add,
                            )
                    ps_s = psum2.tile([128, 128], F32, tag="psS")
                    for d in range(64):
                        nc.tensor.matmul(
                            out=ps_s[:NMETA * CT, :],
                            lhsT=F1v[:, :, :, d],
                            rhs=icat[:, :D],
                            start=(d == 0),
                            stop=(d == 63),
                        )
                    # relu + exp on transposed scores
                    s_r = spool.tile([128, 128], F32, tag="s_r")
                    nc.scalar.activation(s_r[:NMETA * CT, :],
                                         ps_s[:NMETA * CT, :], ACTF.Relu)
                    E_T = spool.tile([128, 128], BF16, tag="E_T")
                    with nc.allow_low_precision(reason="bf16 e"):
                        nc.scalar.activation(E_T[:NMETA * CT, :],
                                             s_r[:NMETA * CT, :], ACTF.Exp)
                    # transpose back via perm -> [128 n, 56 (c m)] and copy
                    ps_e = psum2.tile([128, 64], F32, tag="psE")
                    nc.tensor.matmul(
                        out=ps_e[:, :NMETA * CT + CT],
                        lhsT=E_T[:NMETA * CT, :],
                        rhs=permt[:NMETA * CT, :NMETA * CT + CT],
                        start=True, stop=True,
                    )
                    e_bf = spool.tile([128, CT * NMETA], BF16, tag="e_bf")
                    with nc.allow_low_precision(reason="bf16 e"):
                        nc.scalar.activation(e_bf[:], ps_e[:, :NMETA * CT],
                                             ACTF.Copy)
                else:
                    # scores for the whole tile, (c, m)-ordered
                    scores_t = spool.tile([128, CT * NMETA], F32,
                                          tag="scores_t")
                    sv = scores_t[:].rearrange("p (c m) -> p m c", m=NMETA)
                    for g0, gl in GROUPS:
                        P = ppool.tile([128, NMETA * 4 * D], BF16, tag="P")
                        Pv = P[:].rearrange("p (m g d) -> p m g d",
                                            m=NMETA, g=4)
                        nc.vector.tensor_tensor(
                            out=Pv[:, :, :gl, :],
                            in0=Xv[:, :, g0:g0 + gl, :],
                            in1=wbv.unsqueeze(2).broadcast_to(
                                [128, NMETA, gl, D]),
                            op=ALU.mult,
                        )
                        w = D
                        cur = Pv
                        buf_iter = [f1pool, f2pool, f1pool, f2pool, f1pool,
                                    f2pool]
                        bi = 0
                        with nc.allow_low_precision(reason="bf16 fold tree"):
                            while w > fold_to:
                                h = w // 2
                                Fp = buf_iter[bi].tile(
                                    [128, NMETA * 4 * h], BF16, tag=f"F{h}")
                                bi += 1
                                Fv = Fp[:].rearrange(
                                    "p (m g d) -> p m g d", m=NMETA, g=4)
                                nc.vector.tensor_tensor(
                                    out=Fv[:, :, :gl, :],
                                    in0=cur[:, :, :gl, :h],
                                    in1=cur[:, :, :gl, h:w],
                                    op=ALU.add,
                                )
                                cur = Fv
                                w = h
                        nc.vector.tensor_reduce(
                            out=sv[:, :, g0:g0 + gl],
                            in_=cur[:, :, :gl, :w],
                            axis=mybir.AxisListType.X,
                            op=ALU.add,
                        )

                    # softmax tail: relu+exp on ACT, sums/recip/att on DVE
                    s_r = spool.tile([128, CT * NMETA], F32, tag="s_r")
                    nc.scalar.activation(s_r[:], scores_t[:], ACTF.Relu)
                    e_bf = spool.tile([128, CT * NMETA], BF16, tag="e_bf")
                    with nc.allow_low_precision(reason="bf16 softmax"):
                        nc.scalar.activation(e_bf[:], s_r[:], ACTF.Exp)
                inv = spool.tile([128, CT], F32, tag="inv")
                nc.vector.reciprocal(
                    inv[:], ps_e[:, NMETA * CT:NMETA * CT + CT])
                att_t = apool.tile([128, CT * NMETA], BF16, tag="att_t")
                av = att_t[:].rearrange("p (c m) -> p c m", m=NMETA)
                with nc.allow_low_precision(reason="bf16 att weights"):
                    nc.vector.tensor_tensor(
                        out=av[:],
                        in0=e_bf[:].rearrange("p (c m) -> p c m", m=NMETA),
                        in1=inv[:].unsqueeze(2).broadcast_to(
                            [128, CT, NMETA]),
                        op=ALU.mult,
                    )
                attf_t = apool.tile([128, CT * NMETA], F32, tag="attf_t")
                nc.scalar.activation(attf_t[:], att_t[:], ACTF.Copy)
                tiles[t]["att"] = att_t
                tiles[t]["attf"] = attf_t

            def emit_pool(t):
                Xv = tiles[t]["Xv"]
                att_t = tiles[t]["att"]
                attf_t = tiles[t]["attf"]
                pss = []
                for g0, gl in GROUPS:
                    ps = psum.tile([128, 4 * D], F32, tag="ps")
                    pss.append(ps)
                    for gg in range(gl):
                        c = g0 + gg
                        chunk_idx = t * CT + c
                        dg = dcpool.tile([128, NMETA * D], BF16,
                                         tag=f"dgc{c % 4}")
                        if diag_sched is not None:
                            eng = diag_sched[chunk_idx % len(diag_sched)]
                        else:
                            eng = ("A" if chunk_idx % diag_mod in act_pos
                                   else "P")
                        if eng == "P":
                            nc.gpsimd.local_scatter(
                                dg[:], att_t[:, c * NMETA:(c + 1) * NMETA],
                                sidx[:],
                                channels=128, num_elems=NMETA * D,
                                num_idxs=NMETA)
                        else:
                            # A: all 8 blocks on ACT; D: all on DVE;
                            # H: split ACT (m<4) / DVE (m>=4) to halve
                            # the serial build latency
                            for m in range(NMETA):
                                on_act = (eng == "A" or
                                          (eng == "H" and m < hsplit))
                                if on_act:
                                    nc.scalar.activation(
                                        dg[:, m * D:(m + 1) * D],
                                        icat[:, m * D:(m + 1) * D],
                                        ACTF.Copy,
                                        scale=attf_t[:, c * NMETA + m:
                                                     c * NMETA + m + 1])
                                else:
                                    nc.vector.tensor_scalar(
                                        dg[:, m * D:(m + 1) * D],
                                        icat[:, m * D:(m + 1) * D],
                                        attf_t[:, c * NMETA + m:
                                               c * NMETA + m + 1],
                                        None, ALU.mult)
                        for m in range(NMETA):
                            nc.tensor.matmul(
                                out=ps[:, gg * D:(gg + 1) * D],
                                lhsT=dg[:, m * D:(m + 1) * D],
                                rhs=Xv[:, m, c, :],
                                start=(m == 0),
                                stop=(m == NMETA - 1),
                            )
                tiles[t]["pss"] = pss

            def emit_elu(t):
                outp = tiles[t]["outp"]
                pss = tiles[t]["pss"]
                for gi, (g0, gl) in enumerate(GROUPS):
                    ps = pss[gi]
                    # elu(x)+1 = relu(x) + exp(-relu(-x))  (store out+1)
                    w = gl * D
                    tneg = epool.tile([128, 4 * D], BF16, tag="tneg")
                    nc.scalar.activation(tneg[:, :w], ps[:, :w], ACTF.Relu,
                                         scale=-1.0)
                    e2 = epool.tile([128, 4 * D], BF16, tag="e2")
                    nc.scalar.activation(e2[:, :w], tneg[:, :w], ACTF.Exp,
                                         scale=-1.0)
                    r = epool.tile([128, 4 * D], BF16, tag="r")
                    nc.scalar.activation(r[:, :w], ps[:, :w], ACTF.Relu)
                    with nc.allow_low_precision(reason="bf16 out"):
                        nc.vector.tensor_tensor(
                            out=outp[:, g0 * D:g0 * D + w],
                            in0=r[:, :w], in1=e2[:, :w], op=ALU.add)
                n0 = t * NT
                dsto = out_d[n0:n0 + NT, :].rearrange(
                    "(p c) d -> p (c d)", p=128)
                nc.sync.dma_start(dsto, outp[:])
                del tiles[t]

            for t in range(NTILES):
                emit_front(t)
                if order == "fpe":
                    if t >= lag:
                        emit_pool(t - lag)
                    if t >= lag + 1:
                        emit_elu(t - lag - 1)
                else:
                    if t >= lag + 1:
                        emit_elu(t - lag - 1)
                    if t >= lag:
                        emit_pool(t - lag)
            for t in range(NTILES - lag, NTILES):
                emit_elu(t - 1)
                emit_pool(t)
            emit_elu(NTILES - 1)


def host_inputs(x_np, w_np, nc_pad=NC_PAD):
    """Build per-core input maps from full fp32 inputs."""
    in_maps = []
    wbig = np.ascontiguousarray(
        np.broadcast_to(w_np.T.reshape(1, NMETA * D), (128, NMETA * D))
    ).astype(ml_dtypes.bfloat16)
    sidx = (np.arange(NMETA)[None, :] * D
            + np.arange(128)[:, None]).astype(np.int16)
    # [56, 63]: cols 0-55 permute row (m*CT+c) -> col (c*NMETA+m);
    # cols 56-62 sum over m per chunk c
    perm = np.zeros((NMETA * CT, NMETA * CT + CT), dtype=np.float32)
    for m in range(NMETA):
        for c in range(CT):
            perm[m * CT + c, c * NMETA + m] = 1.0
            perm[m * CT + c, NMETA * CT + c] = 1.0
    perm = perm.astype(ml_dtypes.bfloat16)
    icat = np.ascontiguousarray(
        np.tile(np.eye(128, dtype=np.float32), (1, NMETA))
    ).astype(ml_dtypes.bfloat16)
    nc_raw = x_np.shape[1] // NCORES
    for c in range(NCORES):
        xs = x_np[:, c * nc_raw:(c + 1) * nc_raw, :]
        xp = np.zeros((NMETA, nc_pad, D), dtype=ml_dtypes.bfloat16)
        xp[:, :nc_raw, :] = xs.astype(ml_dtypes.bfloat16)
        in_maps.append({"x": xp, "wb": wbig, "sidx": sidx, "icat": icat,
                        "perm": perm})
    return in_maps


_CACHE = {}


def build(reps=1, **kw):
    key = (reps, tuple(sorted(kw.items())))
    if key in _CACHE:
        return _CACHE[key]
    nc = bacc.Bacc("TRN2", target_bir_lowering=False, debug=False,
                   num_devices=NCORES)
    x = nc.dram_tensor("x", [NMETA, NC_PAD, D], BF16, kind="ExternalInput").ap()
    wb = nc.dram_tensor("wb", [128, NMETA * D], BF16, kind="ExternalInput").ap()
    sidx = nc.dram_tensor("sidx", [128, NMETA], I16,
                          kind="ExternalInput").ap()
    icat = nc.dram_tensor("icat", [128, NMETA * D], BF16,
                          kind="ExternalInput").ap()
    perm = nc.dram_tensor("perm", [NMETA * CT, NMETA * CT + CT], BF16,
                          kind="ExternalInput").ap()
    out = nc.dram_tensor("out", [NC_PAD, D], BF16, kind="ExternalOutput").ap()
    with tile.TileContext(nc) as tc:
        kernel_body(tc, out, x, wb, sidx, icat, perm, reps=reps, **kw)
    nc.compile()
    _CACHE[key] = nc
    return nc


def run(input, W, trace=False, **trace_kwargs):
    x_np = np.asarray(input, dtype=np.float32)
    w_np = np.asarray(W, dtype=np.float32)
    nc = build()
    in_maps = host_inputs(x_np, w_np)
    res = bass_utils.run_bass_kernel_spmd(
        nc, in_maps, core_ids=list(range(NCORES)), trace=trace, **trace_kwargs)
    nc_raw = x_np.shape[1] // NCORES
    full = np.concatenate(
        [np.asarray(res.results[c]["out"][:nc_raw], dtype=np.float32) - 1.0
         for c in range(NCORES)], axis=0)
    return full, res


def kernel(input, W):
    out, _ = run(input, W, trace=False)
    return out


# revision 51
# speedup vs baseline: 1.0007x; 1.0007x over previous
"""MetapathAttentionLayer Trainium2 kernel.

Math (per node n):
    scores[n, m] = sum_d x[m, n, d] * W[d, m]
    att = softmax(relu(scores), axis=m)      (8 metapaths)
    out[n, :] = elu(sum_m att[n, m] * x[m, n, :])

Strategy: shard nodes across 8 cores (data parallel). Per core, n-major
layout [nodes(part), d(free)] in bf16, 14 tiles of 7x128-node chunks,
3-stage software pipeline front(t) / pool(t-1) / elu(t-2):
  - one batched DMA per tile each way (in: all metapaths; out: bf16 of
    out+1, host subtracts the 1)
  - scores: DVE 2x-mode product vs replicated-W + one 2x bf16 fold
    (d 128->64), then 64 PE matmuls against an identity rhs accumulate
    the transposed scores [56=(m,c), 128=n] in f32 PSUM (reduce via
    transpose; Ldweights is free, matmul cost = out columns)
  - softmax in transposed layout on ACT (relu+exp), transposed back to
    n-major via a matmul against a host-uploaded 56x56 permutation;
    sums/recip/att on DVE
  - pooling: PE matmuls with diag(att_m) stationary accumulating over m
    in PSUM; diag built per chunk on Pool (local_scatter) / ACT
    (copy-with-scale-ptr) / DVE (tensor_scalar) per diag_sched to
    balance engine load
  - elu(x)+1 = relu(x) + exp(-relu(-x)) on ACT (bf16) + one 2x DVE add
"""

import os
from contextlib import ExitStack

import numpy as np
import ml_dtypes

import concourse.bass as bass
import concourse.tile as tile
from concourse import bacc, mybir, library_config
import concourse.bass_utils as bass_utils

F32 = mybir.dt.float32
BF16 = mybir.dt.bfloat16
I16 = mybir.dt.int16
ALU = mybir.AluOpType
ACTF = mybir.ActivationFunctionType

NMETA = 8
N = 100000
D = 128
NCORES = 8
NC_RAW = N // NCORES          # 12500 nodes per core
CHUNK = 128                   # nodes per compute chunk (partition dim)
NC_PAD = 12544                # 98 chunks of 128
CT = 7                        # chunks per DMA tile
NT = CT * CHUNK               # 896 nodes per tile
NTILES = NC_PAD // NT         # 14
GROUPS = ((0, 4), (4, 3))     # chunk groups per tile (psum bank = 512 f32)

# of every DIAG_MOD chunks, the first DIAG_POOL go to Pool local_scatter,
# the rest to ACT copy-scale (engine balance)
DIAG_MOD = 7
DIAG_POOL = 8
FOLD_TO = 2                   # fold d down to this width in bf16, then f32 reduce


def kernel_body(tc, out_d, x_d, wb_d, sidx_d, icat_d, perm_d, reps=1,
                diag_mod=DIAG_MOD, diag_pool=DIAG_POOL, fold_to=FOLD_TO,
                lag=1, xbufs=4, order="fpe",
                act_pos=(2, 5), pe_reduce=True,
                diag_sched="PPHPPHPHPHPHPP", dcbufs=6, ebufs=4,
                hsplit=4, pbufs=3, fbufs=3, abufs=5, sbufs=8):
    nc = tc.nc
    with ExitStack() as ctx:
        const = ctx.enter_context(tc.tile_pool(name="const", bufs=1))
        xpool = ctx.enter_context(tc.tile_pool(name="x", bufs=xbufs))
        opool = ctx.enter_context(tc.tile_pool(name="o", bufs=3))
        ppool = ctx.enter_context(tc.tile_pool(name="prod", bufs=pbufs))
        f1pool = ctx.enter_context(tc.tile_pool(name="f1", bufs=fbufs))
        f2pool = ctx.enter_context(tc.tile_pool(name="f2", bufs=3))
        spool = ctx.enter_context(tc.tile_pool(name="smalls", bufs=sbufs))
        apool = ctx.enter_context(tc.tile_pool(name="att", bufs=abufs))
        dcpool = ctx.enter_context(tc.tile_pool(name="diagC", bufs=dcbufs))
        epool = ctx.enter_context(tc.tile_pool(name="elu", bufs=ebufs))
        psum = ctx.enter_context(tc.tile_pool(name="ps", bufs=4, space="PSUM"))
        psum2 = ctx.enter_context(tc.tile_pool(name="ps2", bufs=2,
                                               space="PSUM"))

        wb = const.tile([128, NMETA * D], BF16)
        nc.sync.dma_start(wb[:], wb_d[:])
        sidx = const.tile([128, NMETA], I16)
        nc.sync.dma_start(sidx[:], sidx_d[:])
        icat = const.tile([128, NMETA * D], BF16)
        nc.sync.dma_start(icat[:], icat_d[:])
        permt = const.tile([128, NMETA * CT + CT], BF16)
        nc.sync.dma_start(permt[:NMETA * CT, :], perm_d[:])
        nc.gpsimd.load_library(library_config.local_scatter)

        wbv = wb[:].rearrange("p (m d) -> p m d", m=NMETA)

        for _rep in range(reps):
            # software pipeline over tiles: front = DMA + scores + softmax,
            # back (lag tiles later) = diag + pooling + elu + store.
            tiles = {}

            def emit_front(t):
                n0 = t * NT
                X = xpool.tile([128, NMETA * NT], BF16, tag="X")
                src = x_d[:, n0:n0 + NT, :].rearrange(
                    "m (p c) d -> p m c d", p=128)
                Xv = X[:].rearrange("p (m c d) -> p m c d", m=NMETA, c=CT)
                nc.sync.dma_start(Xv, src)
                outp = opool.tile([128, NT], BF16, tag="outp")
                tiles[t] = {"Xv": Xv, "outp": outp}

                if pe_reduce:
                    # product + one bf16 fold level; then PE reduces the
                    # remaining 64 d-slices into PSUM (f32 accumulate),
                    # yielding transposed scores [56=(m c), 128=n].
                    F1t = f1pool.tile([128, NMETA * CT * 64], BF16,
                                      tag="F1t")
                    F1v = F1t[:].rearrange("p (m c d) -> p m c d",
                                           m=NMETA, c=CT)
                    for g0, gl in GROUPS:
                        P = ppool.tile([128, NMETA * 4 * D], BF16, tag="P")
                        Pv = P[:].rearrange("p (m g d) -> p m g d",
                                            m=NMETA, g=4)
                        nc.vector.tensor_tensor(
                            out=Pv[:, :, :gl, :],
                            in0=Xv[:, :, g0:g0 + gl, :],
                            in1=wbv.unsqueeze(2).broadcast_to(
                                [128, NMETA, gl, D]),
                            op=ALU.mult,
                        )
                        with nc.allow_low_precision(reason="bf16 fold"):
                            nc.vector.tensor_tensor(
                                out=F1v[:, :, g0:g0 + gl, :],
                                in0=Pv[:, :, :gl, :64],
                                in1=Pv[:, :, :gl, 64:],
                                op=ALU.add,
                            )
                    ps_s = psum2.tile([128, 128], F32, tag="psS")
                    for d in range(64):
                        nc.tensor.matmul(
                            out=ps_s[:NMETA * CT, :],
                            lhsT=F1v[:, :, :, d],
                            rhs=icat[:, :D],
                            start=(d == 0),
                            stop=(d == 63),
                        )
                    # relu + exp on transposed scores
                    s_r = spool.tile([128, 128], F32, tag="s_r")
                    nc.scalar.activation(s_r[:NMETA * CT, :],
                                         ps_s[:NMETA * CT, :], ACTF.Relu)
                    E_T = spool.tile([128, 128], BF16, tag="E_T")
                    with nc.allow_low_precision(reason="bf16 e"):
                        nc.scalar.activation(E_T[:NMETA * CT, :],
                                             s_r[:NMETA * CT, :], ACTF.Exp)
                    # transpose back via perm -> [128 n, 56 (c m)] and copy
                    ps_e = psum2.tile([128, 64], F32, tag="psE")
                    nc.tensor.matmul(
                        out=ps_e[:, :NMETA * CT + CT],
                        lhsT=E_T[:NMETA * CT, :],
                        rhs=permt[:NMETA * CT, :NMETA * CT + CT],
                        start=True, stop=True,
                    )
                    e_bf = spool.tile([128, CT * NMETA], BF16, tag="e_bf")
                    with nc.allow_low_precision(reason="bf16 e"):
                        nc.scalar.activation(e_bf[:], ps_e[:, :NMETA * CT],
                                             ACTF.Copy)
                else:
                    # scores for the whole tile, (c, m)-ordered
                    scores_t = spool.tile([128, CT * NMETA], F32,
                                          tag="scores_t")
                    sv = scores_t[:].rearrange("p (c m) -> p m c", m=NMETA)
                    for g0, gl in GROUPS:
                        P = ppool.tile([128, NMETA * 4 * D], BF16, tag="P")
                        Pv = P[:].rearrange("p (m g d) -> p m g d",
                                            m=NMETA, g=4)
                        nc.vector.tensor_tensor(
                            out=Pv[:, :, :gl, :],
                            in0=Xv[:, :, g0:g0 + gl, :],
                            in1=wbv.unsqueeze(2).broadcast_to(
                                [128, NMETA, gl, D]),
                            op=ALU.mult,
                        )
                        w = D
                        cur = Pv
                        buf_iter = [f1pool, f2pool, f1pool, f2pool, f1pool,
                                    f2pool]
                        bi = 0
                        with nc.allow_low_precision(reason="bf16 fold tree"):
                            while w > fold_to:
                                h = w // 2
                                Fp = buf_iter[bi].tile(
                                    [128, NMETA * 4 * h], BF16, tag=f"F{h}")
                                bi += 1
                                Fv = Fp[:].rearrange(
                                    "p (m g d) -> p m g d", m=NMETA, g=4)
                                nc.vector.tensor_tensor(
                                    out=Fv[:, :, :gl, :],
                                    in0=cur[:, :, :gl, :h],
                                    in1=cur[:, :, :gl, h:w],
                                    op=ALU.add,
                                )
                                cur = Fv
                                w = h
                        nc.vector.tensor_reduce(
                            out=sv[:, :, g0:g0 + gl],
                            in_=cur[:, :, :gl, :w],
                            axis=mybir.AxisListType.X,
                            op=ALU.add,
                        )

                    # softmax tail: relu+exp on ACT, sums/recip/att on DVE
                    s_r = spool.tile([128, CT * NMETA], F32, tag="s_r")
                    nc.scalar.activation(s_r[:], scores_t[:], ACTF.Relu)
                    e_bf = spool.tile([128, CT * NMETA], BF16, tag="e_bf")
                    with nc.allow_low_precision(reason="bf16 softmax"):
                        nc.scalar.activation(e_bf[:], s_r[:], ACTF.Exp)
                inv = spool.tile([128, CT], F32, tag="inv")
                nc.vector.reciprocal(
                    inv[:], ps_e[:, NMETA * CT:NMETA * CT + CT])
                att_t = apool.tile([128, CT * NMETA], BF16, tag="att_t")
                av = att_t[:].rearrange("p (c m) -> p c m", m=NMETA)
                with nc.allow_low_precision(reason="bf16 att weights"):
                    nc.vector.tensor_tensor(
                        out=av[:],
                        in0=e_bf[:].rearrange("p (c m) -> p c m", m=NMETA),
                        in1=inv[:].unsqueeze(2).broadcast_to(
                            [128, CT, NMETA]),
                        op=ALU.mult,
                    )
                attf_t = apool.tile([128, CT * NMETA], F32, tag="attf_t")
                nc.scalar.activation(attf_t[:], att_t[:], ACTF.Copy)
                tiles[t]["att"] = att_t
                tiles[t]["attf"] = attf_t

            def emit_pool(t):
                Xv = tiles[t]["Xv"]
                att_t = tiles[t]["att"]
                attf_t = tiles[t]["attf"]
                pss = []
                for g0, gl in GROUPS:
                    ps = psum.tile([128, 4 * D], F32, tag="ps")
                    pss.append(ps)
                    for gg in range(gl):
                        c = g0 + gg
                        chunk_idx = t * CT + c
                        dg = dcpool.tile([128, NMETA * D], BF16,
                                         tag=f"dgc{c % 4}")
                        if diag_sched is not None:
                            eng = diag_sched[chunk_idx % len(diag_sched)]
                        else:
                            eng = ("A" if chunk_idx % diag_mod in act_pos
                                   else "P")
                        if eng == "P":
                            nc.gpsimd.local_scatter(
                                dg[:], att_t[:, c * NMETA:(c + 1) * NMETA],
                                sidx[:],
                                channels=128, num_elems=NMETA * D,
                                num_idxs=NMETA)
                        else:
                            # A: all 8 blocks on ACT; D: all on DVE;
                            # H: split ACT (m<4) / DVE (m>=4) to halve
                            # the serial build latency
                            for m in range(NMETA):
                                on_act = (eng == "A" or
                                          (eng == "H" and m < hsplit))
                                if on_act:
                                    nc.scalar.activation(
                                        dg[:, m * D:(m + 1) * D],
                                        icat[:, m * D:(m + 1) * D],
                                        ACTF.Copy,
                                        scale=attf_t[:, c * NMETA + m:
                                                     c * NMETA + m + 1])
                                else:
                                    nc.vector.tensor_scalar(
                                        dg[:, m * D:(m + 1) * D],
                                        icat[:, m * D:(m + 1) * D],
                                        attf_t[:, c * NMETA + m:
                                               c * NMETA + m + 1],
                                        None, ALU.mult)
                        for m in range(NMETA):
                            nc.tensor.matmul(
                                out=ps[:, gg * D:(gg + 1) * D],
                                lhsT=dg[:, m * D:(m + 1) * D],
                                rhs=Xv[:, m, c, :],
                                start=(m == 0),
                                stop=(m == NMETA - 1),
                            )
                tiles[t]["pss"] = pss

            def emit_elu(t):
                outp = tiles[t]["outp"]
                pss = tiles[t]["pss"]
                for gi, (g0, gl) in enumerate(GROUPS):
                    ps = pss[gi]
                    # elu(x)+1 = relu(x) + exp(-relu(-x))  (store out+1)
                    w = gl * D
                    tneg = epool.tile([128, 4 * D], BF16, tag="tneg")
                    nc.scalar.activation(tneg[:, :w], ps[:, :w], ACTF.Relu,
                                         scale=-1.0)
                    e2 = epool.tile([128, 4 * D], BF16, tag="e2")
                    nc.scalar.activation(e2[:, :w], tneg[:, :w], ACTF.Exp,
                                         scale=-1.0)
                    r = epool.tile([128, 4 * D], BF16, tag="r")
                    nc.scalar.activation(r[:, :w], ps[:, :w], ACTF.Relu)
                    with nc.allow_low_precision(reason="bf16 out"):
                        nc.vector.tensor_tensor(
                            out=outp[:, g0 * D:g0 * D + w],
                            in0=r[:, :w], in1=e2[:, :w], op=ALU.add)
                n0 = t * NT
                dsto = out_d[n0:n0 + NT, :].rearrange(
                    "(p c) d -> p (c d)", p=128)
                nc.sync.dma_start(dsto, outp[:])
                del tiles[t]

            for t in range(NTILES):
                emit_front(t)
                if order == "fpe":
                    if t >= lag:
                        emit_pool(t - lag)
                    if t >= lag + 1:
                        emit_elu(t - lag - 1)
                else:
                    if t >= lag + 1:
                        emit_elu(t - lag - 1)
                    if t >= lag:
                        emit_pool(t - lag)
            for t in range(NTILES - lag, NTILES):
                emit_elu(t - 1)
                emit_pool(t)
            emit_elu(NTILES - 1)


def host_inputs(x_np, w_np, nc_pad=NC_PAD):
    """Build per-core input maps from full fp32 inputs."""
    in_maps = []
    wbig = np.ascontiguousarray(
        np.broadcast_to(w_np.T.reshape(1, NMETA * D), (128, NMETA * D))
    ).astype(ml_dtypes.bfloat16)
    sidx = (np.arange(NMETA)[None, :] * D
            + np.arange(128)[:, None]).astype(np.int16)
    # [56, 63]: cols 0-55 permute row (m*CT+c) -> col (c*NMETA+m);
    # cols 56-62 sum over m per chunk c
    perm = np.zeros((NMETA * CT, NMETA * CT + CT), dtype=np.float32)
    for m in range(NMETA):
        for c in range(CT):
            perm[m * CT + c, c * NMETA + m] = 1.0
            perm[m * CT + c, NMETA * CT + c] = 1.0
    perm = perm.astype(ml_dtypes.bfloat16)
    icat = np.ascontiguousarray(
        np.tile(np.eye(128, dtype=np.float32), (1, NMETA))
    ).astype(ml_dtypes.bfloat16)
    nc_raw = x_np.shape[1] // NCORES
    for c in range(NCORES):
        xs = x_np[:, c * nc_raw:(c + 1) * nc_raw, :]
        xp = np.zeros((NMETA, nc_pad, D), dtype=ml_dtypes.bfloat16)
        xp[:, :nc_raw, :] = xs.astype(ml_dtypes.bfloat16)
        in_maps.append({"x": xp, "wb": wbig, "sidx": sidx, "icat": icat,
                        "perm": perm})
    return in_maps


_CACHE = {}


def build(reps=1, **kw):
    key = (reps, tuple(sorted(kw.items())))
    if key in _CACHE:
        return _CACHE[key]
    nc = bacc.Bacc("TRN2", target_bir_lowering=False, debug=False,
                   num_devices=NCORES)
    x = nc.dram_tensor("x", [NMETA, NC_PAD, D], BF16, kind="ExternalInput").ap()
    wb = nc.dram_tensor("wb", [128, NMETA * D], BF16, kind="ExternalInput").ap()
    sidx = nc.dram_tensor("sidx", [128, NMETA], I16,
                          kind="ExternalInput").ap()
    icat = nc.dram_tensor("icat", [128, NMETA * D], BF16,
                          kind="ExternalInput").ap()
    perm = nc.dram_tensor("perm", [NMETA * CT, NMETA * CT + CT], BF16,
                          kind="ExternalInput").ap()
    out = nc.dram_tensor("out", [NC_PAD, D], BF16, kind="ExternalOutput").ap()
    with tile.TileContext(nc) as tc:
        kernel_body(tc, out, x, wb, sidx, icat, perm, reps=reps, **kw)
    nc.compile()
    _CACHE[key] = nc
    return nc


def run(input, W, trace=False, **trace_kwargs):
    x_np = np.asarray(input, dtype=np.float32)
    w_np = np.asarray(W, dtype=np.float32)
    nc = build()
    in_maps = host_inputs(x_np, w_np)
    res = bass_utils.run_bass_kernel_spmd(
        nc, in_maps, core_ids=list(range(NCORES)), trace=trace, **trace_kwargs)
    nc_raw = x_np.shape[1] // NCORES
    full = np.concatenate(
        [np.asarray(res.results[c]["out"][:nc_raw], dtype=np.float32) - 1.0
         for c in range(NCORES)], axis=0)
    return full, res


def kernel(input, W):
    out, _ = run(input, W, trace=False)
    return out


# revision 52
# speedup vs baseline: 1.0022x; 1.0015x over previous
"""MetapathAttentionLayer Trainium2 kernel.

Math (per node n):
    scores[n, m] = sum_d x[m, n, d] * W[d, m]
    att = softmax(relu(scores), axis=m)      (8 metapaths)
    out[n, :] = elu(sum_m att[n, m] * x[m, n, :])

Strategy: shard nodes across 8 cores (data parallel). Per core, n-major
layout [nodes(part), d(free)] in bf16, 14 tiles of 7x128-node chunks,
3-stage software pipeline front(t) / pool(t-1) / elu(t-2):
  - one batched DMA per tile each way (in: all metapaths; out: bf16 of
    out+1, host subtracts the 1)
  - scores: DVE 2x-mode product vs replicated-W + one 2x bf16 fold
    (d 128->64), then 64 PE matmuls against an identity rhs accumulate
    the transposed scores [56=(m,c), 128=n] in f32 PSUM (reduce via
    transpose; Ldweights is free, matmul cost = out columns)
  - softmax in transposed layout on ACT (relu+exp), transposed back to
    n-major via a matmul against a host-uploaded 56x56 permutation;
    sums/recip/att on DVE
  - pooling: PE matmuls with diag(att_m) stationary accumulating over m
    in PSUM; diag built per chunk on Pool (local_scatter) / ACT
    (copy-with-scale-ptr) / DVE (tensor_scalar) per diag_sched to
    balance engine load
  - elu(x)+1 = relu(x) + exp(-relu(-x)) on ACT (bf16) + one 2x DVE add
"""

import os
from contextlib import ExitStack

import numpy as np
import ml_dtypes

import concourse.bass as bass
import concourse.tile as tile
from concourse import bacc, mybir, library_config
import concourse.bass_utils as bass_utils

F32 = mybir.dt.float32
BF16 = mybir.dt.bfloat16
I16 = mybir.dt.int16
ALU = mybir.AluOpType
ACTF = mybir.ActivationFunctionType

NMETA = 8
N = 100000
D = 128
NCORES = 8
NC_RAW = N // NCORES          # 12500 nodes per core
CHUNK = 128                   # nodes per compute chunk (partition dim)
NC_PAD = 12544                # 98 chunks of 128
CT = 7                        # chunks per DMA tile
NT = CT * CHUNK               # 896 nodes per tile
NTILES = NC_PAD // NT         # 14
GROUPS = ((0, 4), (4, 3))     # chunk groups per tile (psum bank = 512 f32)

# of every DIAG_MOD chunks, the first DIAG_POOL go to Pool local_scatter,
# the rest to ACT copy-scale (engine balance)
DIAG_MOD = 7
DIAG_POOL = 8
FOLD_TO = 2                   # fold d down to this width in bf16, then f32 reduce


def kernel_body(tc, out_d, x_d, wb_d, sidx_d, icat_d, perm_d, reps=1,
                diag_mod=DIAG_MOD, diag_pool=DIAG_POOL, fold_to=FOLD_TO,
                lag=1, xbufs=4, order="fpe",
                act_pos=(2, 5), pe_reduce=True,
                diag_sched="PPHPPHPHPHPHPP", dcbufs=6, ebufs=4,
                hsplit=4, pbufs=3, fbufs=3, abufs=5, sbufs=8):
    nc = tc.nc
    with ExitStack() as ctx:
        const = ctx.enter_context(tc.tile_pool(name="const", bufs=1))
        xpool = ctx.enter_context(tc.tile_pool(name="x", bufs=xbufs))
        opool = ctx.enter_context(tc.tile_pool(name="o", bufs=3))
        ppool = ctx.enter_context(tc.tile_pool(name="prod", bufs=pbufs))
        f1pool = ctx.enter_context(tc.tile_pool(name="f1", bufs=fbufs))
        f2pool = ctx.enter_context(tc.tile_pool(name="f2", bufs=3))
        spool = ctx.enter_context(tc.tile_pool(name="smalls", bufs=sbufs))
        apool = ctx.enter_context(tc.tile_pool(name="att", bufs=abufs))
        dcpool = ctx.enter_context(tc.tile_pool(name="diagC", bufs=dcbufs))
        epool = ctx.enter_context(tc.tile_pool(name="elu", bufs=ebufs))
        psum = ctx.enter_context(tc.tile_pool(name="ps", bufs=4, space="PSUM"))
        psum2 = ctx.enter_context(tc.tile_pool(name="ps2", bufs=2,
                                               space="PSUM"))

        wb = const.tile([128, NMETA * D], BF16)
        nc.sync.dma_start(wb[:], wb_d[:])
        sidx = const.tile([128, NMETA], I16)
        nc.sync.dma_start(sidx[:], sidx_d[:])
        icat = const.tile([128, NMETA * D], BF16)
        nc.sync.dma_start(icat[:], icat_d[:])
        permt = const.tile([128, NMETA * CT + CT], BF16)
        nc.sync.dma_start(permt[:NMETA * CT, :], perm_d[:])
        nc.gpsimd.load_library(library_config.local_scatter)

        wbv = wb[:].rearrange("p (m d) -> p m d", m=NMETA)

        for _rep in range(reps):
            # software pipeline over tiles: front = DMA + scores + softmax,
            # back (lag tiles later) = diag + pooling + elu + store.
            tiles = {}

            def emit_front(t):
                n0 = t * NT
                X = xpool.tile([128, NMETA * NT], BF16, tag="X")
                src = x_d[:, n0:n0 + NT, :].rearrange(
                    "m (p c) d -> p m c d", p=128)
                Xv = X[:].rearrange("p (m c d) -> p m c d", m=NMETA, c=CT)
                nc.sync.dma_start(Xv, src)
                outp = opool.tile([128, NT], BF16, tag="outp")
                tiles[t] = {"Xv": Xv, "outp": outp}

                if pe_reduce:
                    # product + one bf16 fold level; then PE reduces the
                    # remaining 64 d-slices into PSUM (f32 accumulate),
                    # yielding transposed scores [56=(m c), 128=n].
                    F1t = f1pool.tile([128, NMETA * CT * 64], BF16,
                                      tag="F1t")
                    F1v = F1t[:].rearrange("p (m c d) -> p m c d",
                                           m=NMETA, c=CT)
                    for g0, gl in GROUPS:
                        P = ppool.tile([128, NMETA * 4 * D], BF16, tag="P")
                        Pv = P[:].rearrange("p (m g d) -> p m g d",
                                            m=NMETA, g=4)
                        nc.vector.tensor_tensor(
                            out=Pv[:, :, :gl, :],
                            in0=Xv[:, :, g0:g0 + gl, :],
                            in1=wbv.unsqueeze(2).broadcast_to(
                                [128, NMETA, gl, D]),
                            op=ALU.mult,
                        )
                        with nc.allow_low_precision(reason="bf16 fold"):
                            nc.vector.tensor_tensor(
                                out=F1v[:, :, g0:g0 + gl, :],
                                in0=Pv[:, :, :gl, :64],
                                in1=Pv[:, :, :gl, 64:],
                                op=ALU.add,
                            )
                    ps_s = psum2.tile([128, 128], F32, tag="psS")
                    for d in range(64):
                        nc.tensor.matmul(
                            out=ps_s[:NMETA * CT, :],
                            lhsT=F1v[:, :, :, d],
                            rhs=icat[:, :D],
                            start=(d == 0),
                            stop=(d == 63),
                        )
                    # relu + exp on transposed scores
                    s_r = spool.tile([128, 128], F32, tag="s_r")
                    nc.scalar.activation(s_r[:NMETA * CT, :],
                                         ps_s[:NMETA * CT, :], ACTF.Relu)
                    E_T = spool.tile([128, 128], BF16, tag="E_T")
                    with nc.allow_low_precision(reason="bf16 e"):
                        nc.scalar.activation(E_T[:NMETA * CT, :],
                                             s_r[:NMETA * CT, :], ACTF.Exp)
                    # transpose back via perm -> [128 n, 56 (c m)] and copy
                    ps_e = psum2.tile([128, 64], F32, tag="psE")
                    nc.tensor.matmul(
                        out=ps_e[:, :NMETA * CT + CT],
                        lhsT=E_T[:NMETA * CT, :],
                        rhs=permt[:NMETA * CT, :NMETA * CT + CT],
                        start=True, stop=True,
                    )
                    e_bf = spool.tile([128, CT * NMETA], BF16, tag="e_bf")
                    with nc.allow_low_precision(reason="bf16 e"):
                        nc.scalar.activation(e_bf[:], ps_e[:, :NMETA * CT],
                                             ACTF.Copy)
                else:
                    # scores for the whole tile, (c, m)-ordered
                    scores_t = spool.tile([128, CT * NMETA], F32,
                                          tag="scores_t")
                    sv = scores_t[:].rearrange("p (c m) -> p m c", m=NMETA)
                    for g0, gl in GROUPS:
                        P = ppool.tile([128, NMETA * 4 * D], BF16, tag="P")
                        Pv = P[:].rearrange("p (m g d) -> p m g d",
                                            m=NMETA, g=4)
                        nc.vector.tensor_tensor(
                            out=Pv[:, :, :gl, :],
                            in0=Xv[:, :, g0:g0 + gl, :],
                            in1=wbv.unsqueeze(2).broadcast_to(
                                [128, NMETA, gl, D]),
                            op=ALU.mult,
                        )
                        w = D
                        cur = Pv
                        buf_iter = [f1pool, f2pool, f1pool, f2pool, f1pool,
                                    f2pool]
                        bi = 0
                        with nc.allow_low_precision(reason="bf16 fold tree"):
                            while w > fold_to:
                                h = w // 2
                                Fp = buf_iter[bi].tile(
                                    [128, NMETA * 4 * h], BF16, tag=f"F{h}")
                                bi += 1
                                Fv = Fp[:].rearrange(
                                    "p (m g d) -> p m g d", m=NMETA, g=4)
                                nc.vector.tensor_tensor(
                                    out=Fv[:, :, :gl, :],
                                    in0=cur[:, :, :gl, :h],
                                    in1=cur[:, :, :gl, h:w],
                                    op=ALU.add,
                                )
                                cur = Fv
                                w = h
                        nc.vector.tensor_reduce(
                            out=sv[:, :, g0:g0 + gl],
                            in_=cur[:, :, :gl, :w],
                            axis=mybir.AxisListType.X,
                            op=ALU.add,
                        )

                    # softmax tail: relu+exp on ACT, sums/recip/att on DVE
                    s_r = spool.tile([128, CT * NMETA], F32, tag="s_r")
                    nc.scalar.activation(s_r[:], scores_t[:], ACTF.Relu)
                    e_bf = spool.tile([128, CT * NMETA], BF16, tag="e_bf")
                    with nc.allow_low_precision(reason="bf16 softmax"):
                        nc.scalar.activation(e_bf[:], s_r[:], ACTF.Exp)
                inv = spool.tile([128, CT], F32, tag="inv")
                nc.vector.reciprocal(
                    inv[:], ps_e[:, NMETA * CT:NMETA * CT + CT])
                att_t = apool.tile([128, CT * NMETA], BF16, tag="att_t")
                av = att_t[:].rearrange("p (c m) -> p c m", m=NMETA)
                with nc.allow_low_precision(reason="bf16 att weights"):
                    nc.vector.tensor_tensor(
                        out=av[:],
                        in0=e_bf[:].rearrange("p (c m) -> p c m", m=NMETA),
                        in1=inv[:].unsqueeze(2).broadcast_to(
                            [128, CT, NMETA]),
                        op=ALU.mult,
                    )
                attf_t = apool.tile([128, CT * NMETA], F32, tag="attf_t")
                nc.scalar.activation(attf_t[:], att_t[:], ACTF.Copy)
                tiles[t]["att"] = att_t
                tiles[t]["attf"] = attf_t

            def emit_pool(t):
                Xv = tiles[t]["Xv"]
                att_t = tiles[t]["att"]
                attf_t = tiles[t]["attf"]
                pss = []
                for g0, gl in GROUPS:
                    ps = psum.tile([128, 4 * D], F32, tag="ps")
                    pss.append(ps)
                    for gg in range(gl):
                        c = g0 + gg
                        chunk_idx = t * CT + c
                        dg = dcpool.tile([128, NMETA * D], BF16,
                                         tag=f"dgc{c % 4}")
                        if diag_sched is not None:
                            eng = diag_sched[chunk_idx % len(diag_sched)]
                        else:
                            eng = ("A" if chunk_idx % diag_mod in act_pos
                                   else "P")
                        if eng == "P":
                            nc.gpsimd.local_scatter(
                                dg[:], att_t[:, c * NMETA:(c + 1) * NMETA],
                                sidx[:],
                                channels=128, num_elems=NMETA * D,
                                num_idxs=NMETA)
                        else:
                            # A: all 8 blocks on ACT; D: all on DVE;
                            # H: split ACT (m<4) / DVE (m>=4) to halve
                            # the serial build latency
                            for m in range(NMETA):
                                on_act = (eng == "A" or
                                          (eng == "H" and m < hsplit))
                                if on_act:
                                    nc.scalar.activation(
                                        dg[:, m * D:(m + 1) * D],
                                        icat[:, m * D:(m + 1) * D],
                                        ACTF.Copy,
                                        scale=attf_t[:, c * NMETA + m:
                                                     c * NMETA + m + 1])
                                else:
                                    nc.vector.tensor_scalar(
                                        dg[:, m * D:(m + 1) * D],
                                        icat[:, m * D:(m + 1) * D],
                                        attf_t[:, c * NMETA + m:
                                               c * NMETA + m + 1],
                                        None, ALU.mult)
                        for m in range(NMETA):
                            nc.tensor.matmul(
                                out=ps[:, gg * D:(gg + 1) * D],
                                lhsT=dg[:, m * D:(m + 1) * D],
                                rhs=Xv[:, m, c, :],
                                start=(m == 0),
                                stop=(m == NMETA - 1),
                            )
                tiles[t]["pss"] = pss

            def emit_elu(t):
                outp = tiles[t]["outp"]
                pss = tiles[t]["pss"]
                for gi, (g0, gl) in enumerate(GROUPS):
                    ps = pss[gi]
                    # elu(x)+1 = relu(x) + exp(-relu(-x))  (store out+1)
                    w = gl * D
                    tneg = epool.tile([128, 4 * D], BF16, tag="tneg")
                    nc.scalar.activation(tneg[:, :w], ps[:, :w], ACTF.Relu,
                                         scale=-1.0)
                    e2 = epool.tile([128, 4 * D], BF16, tag="e2")
                    nc.scalar.activation(e2[:, :w], tneg[:, :w], ACTF.Exp,
                                         scale=-1.0)
                    r = epool.tile([128, 4 * D], BF16, tag="r")
                    nc.scalar.activation(r[:, :w], ps[:, :w], ACTF.Relu)
                    with nc.allow_low_precision(reason="bf16 out"):
                        nc.vector.tensor_tensor(
                            out=outp[:, g0 * D:g0 * D + w],
                            in0=r[:, :w], in1=e2[:, :w], op=ALU.add)
                n0 = t * NT
                dsto = out_d[n0:n0 + NT, :].rearrange(
                    "(p c) d -> p (c d)", p=128)
                nc.sync.dma_start(dsto, outp[:])
                del tiles[t]

            for t in range(NTILES):
                emit_front(t)
                if order == "fpe":
                    if t >= lag:
                        emit_pool(t - lag)
                    if t >= lag + 1:
                        emit_elu(t - lag - 1)
                else:
                    if t >= lag + 1:
                        emit_elu(t - lag - 1)
                    if t >= lag:
                        emit_pool(t - lag)
            for t in range(NTILES - lag, NTILES):
                emit_pool(t)
                emit_elu(t - 1)
            emit_elu(NTILES - 1)


def host_inputs(x_np, w_np, nc_pad=NC_PAD):
    """Build per-core input maps from full fp32 inputs."""
    in_maps = []
    wbig = np.ascontiguousarray(
        np.broadcast_to(w_np.T.reshape(1, NMETA * D), (128, NMETA * D))
    ).astype(ml_dtypes.bfloat16)
    sidx = (np.arange(NMETA)[None, :] * D
            + np.arange(128)[:, None]).astype(np.int16)
    # [56, 63]: cols 0-55 permute row (m*CT+c) -> col (c*NMETA+m);
    # cols 56-62 sum over m per chunk c
    perm = np.zeros((NMETA * CT, NMETA * CT + CT), dtype=np.float32)
    for m in range(NMETA):
        for c in range(CT):
            perm[m * CT + c, c * NMETA + m] = 1.0
            perm[m * CT + c, NMETA * CT + c] = 1.0
    perm = perm.astype(ml_dtypes.bfloat16)
    icat = np.ascontiguousarray(
        np.tile(np.eye(128, dtype=np.float32), (1, NMETA))
    ).astype(ml_dtypes.bfloat16)
    nc_raw = x_np.shape[1] // NCORES
    for c in range(NCORES):
        xs = x_np[:, c * nc_raw:(c + 1) * nc_raw, :]
        xp = np.zeros((NMETA, nc_pad, D), dtype=ml_dtypes.bfloat16)
        xp[:, :nc_raw, :] = xs.astype(ml_dtypes.bfloat16)
        in_maps.append({"x": xp, "wb": wbig, "sidx": sidx, "icat": icat,
                        "perm": perm})
    return in_maps


_CACHE = {}


def build(reps=1, **kw):
    key = (reps, tuple(sorted(kw.items())))
    if key in _CACHE:
        return _CACHE[key]
    nc = bacc.Bacc("TRN2", target_bir_lowering=False, debug=False,
                   num_devices=NCORES)
    x = nc.dram_tensor("x", [NMETA, NC_PAD, D], BF16, kind="ExternalInput").ap()
    wb = nc.dram_tensor("wb", [128, NMETA * D], BF16, kind="ExternalInput").ap()
    sidx = nc.dram_tensor("sidx", [128, NMETA], I16,
                          kind="ExternalInput").ap()
    icat = nc.dram_tensor("icat", [128, NMETA * D], BF16,
                          kind="ExternalInput").ap()
    perm = nc.dram_tensor("perm", [NMETA * CT, NMETA * CT + CT], BF16,
                          kind="ExternalInput").ap()
    out = nc.dram_tensor("out", [NC_PAD, D], BF16, kind="ExternalOutput").ap()
    with tile.TileContext(nc) as tc:
        kernel_body(tc, out, x, wb, sidx, icat, perm, reps=reps, **kw)
    nc.compile()
    _CACHE[key] = nc
    return nc


def run(input, W, trace=False, **trace_kwargs):
    x_np = np.asarray(input, dtype=np.float32)
    w_np = np.asarray(W, dtype=np.float32)
    nc = build()
    in_maps = host_inputs(x_np, w_np)
    res = bass_utils.run_bass_kernel_spmd(
        nc, in_maps, core_ids=list(range(NCORES)), trace=trace, **trace_kwargs)
    nc_raw = x_np.shape[1] // NCORES
    full = np.concatenate(
        [np.asarray(res.results[c]["out"][:nc_raw], dtype=np.float32) - 1.0
         for c in range(NCORES)], axis=0)
    return full, res


def kernel(input, W):
    out, _ = run(input, W, trace=False)
    return out


# revision 54
# speedup vs baseline: 1.0025x; 1.0002x over previous
"""MetapathAttentionLayer Trainium2 kernel.

Math (per node n):
    scores[n, m] = sum_d x[m, n, d] * W[d, m]
    att = softmax(relu(scores), axis=m)      (8 metapaths)
    out[n, :] = elu(sum_m att[n, m] * x[m, n, :])

Strategy: shard nodes across 8 cores (data parallel). Per core, n-major
layout [nodes(part), d(free)] in bf16, 14 tiles of 7x128-node chunks,
3-stage software pipeline front(t) / pool(t-1) / elu(t-2):
  - one batched DMA per tile each way (in: all metapaths; out: bf16 of
    out+1, host subtracts the 1)
  - scores: DVE 2x-mode product vs replicated-W + one 2x bf16 fold
    (d 128->64), then 64 PE matmuls against an identity rhs accumulate
    the transposed scores [56=(m,c), 128=n] in f32 PSUM (reduce via
    transpose; Ldweights is free, matmul cost = out columns)
  - softmax in transposed layout on ACT (relu+exp), transposed back to
    n-major via a matmul against a host-uploaded 56x56 permutation;
    sums/recip/att on DVE
  - pooling: PE matmuls with diag(att_m) stationary accumulating over m
    in PSUM; diag built per chunk on Pool (local_scatter) / ACT
    (copy-with-scale-ptr) / DVE (tensor_scalar) per diag_sched to
    balance engine load
  - elu(x)+1 = relu(x) + exp(-relu(-x)) on ACT (bf16) + one 2x DVE add
"""

import os
from contextlib import ExitStack

import numpy as np
import ml_dtypes

import concourse.bass as bass
import concourse.tile as tile
from concourse import bacc, mybir, library_config
import concourse.bass_utils as bass_utils

F32 = mybir.dt.float32
BF16 = mybir.dt.bfloat16
I16 = mybir.dt.int16
ALU = mybir.AluOpType
ACTF = mybir.ActivationFunctionType

NMETA = 8
N = 100000
D = 128
NCORES = 8
NC_RAW = N // NCORES          # 12500 nodes per core
CHUNK = 128                   # nodes per compute chunk (partition dim)
NC_PAD = 12544                # 98 chunks of 128
CT = 7                        # chunks per DMA tile
NT = CT * CHUNK               # 896 nodes per tile
NTILES = NC_PAD // NT         # 14
GROUPS = ((0, 4), (4, 3))     # chunk groups per tile (psum bank = 512 f32)

# of every DIAG_MOD chunks, the first DIAG_POOL go to Pool local_scatter,
# the rest to ACT copy-scale (engine balance)
DIAG_MOD = 7
DIAG_POOL = 8
FOLD_TO = 2                   # fold d down to this width in bf16, then f32 reduce


def kernel_body(tc, out_d, x_d, wb_d, sidx_d, icat_d, perm_d, reps=1,
                diag_mod=DIAG_MOD, diag_pool=DIAG_POOL, fold_to=FOLD_TO,
                lag=1, xbufs=4, order="fpe",
                act_pos=(2, 5), pe_reduce=True,
                diag_sched="PPHPPHPHPHPHPP", dcbufs=6, ebufs=4,
                hsplit=4, pbufs=3, fbufs=3, abufs=5, sbufs=8,
                halt=True):
    nc = tc.nc
    with ExitStack() as ctx:
        const = ctx.enter_context(tc.tile_pool(name="const", bufs=1))
        xpool = ctx.enter_context(tc.tile_pool(name="x", bufs=xbufs))
        opool = ctx.enter_context(tc.tile_pool(name="o", bufs=3))
        ppool = ctx.enter_context(tc.tile_pool(name="prod", bufs=pbufs))
        f1pool = ctx.enter_context(tc.tile_pool(name="f1", bufs=fbufs))
        f2pool = ctx.enter_context(tc.tile_pool(name="f2", bufs=3))
        spool = ctx.enter_context(tc.tile_pool(name="smalls", bufs=sbufs))
        apool = ctx.enter_context(tc.tile_pool(name="att", bufs=abufs))
        dcpool = ctx.enter_context(tc.tile_pool(name="diagC", bufs=dcbufs))
        epool = ctx.enter_context(tc.tile_pool(name="elu", bufs=ebufs))
        psum = ctx.enter_context(tc.tile_pool(name="ps", bufs=4, space="PSUM"))
        psum2 = ctx.enter_context(tc.tile_pool(name="ps2", bufs=2,
                                               space="PSUM"))

        wb = const.tile([128, NMETA * D], BF16)
        nc.sync.dma_start(wb[:], wb_d[:])
        sidx = const.tile([128, NMETA], I16)
        nc.sync.dma_start(sidx[:], sidx_d[:])
        icat = const.tile([128, NMETA * D], BF16)
        nc.sync.dma_start(icat[:], icat_d[:])
        permt = const.tile([128, NMETA * CT + CT], BF16)
        nc.sync.dma_start(permt[:NMETA * CT, :], perm_d[:])
        nc.gpsimd.load_library(library_config.local_scatter)

        wbv = wb[:].rearrange("p (m d) -> p m d", m=NMETA)

        for _rep in range(reps):
            # software pipeline over tiles: front = DMA + scores + softmax,
            # back (lag tiles later) = diag + pooling + elu + store.
            tiles = {}

            def emit_front(t):
                n0 = t * NT
                X = xpool.tile([128, NMETA * NT], BF16, tag="X")
                src = x_d[:, n0:n0 + NT, :].rearrange(
                    "m (p c) d -> p m c d", p=128)
                Xv = X[:].rearrange("p (m c d) -> p m c d", m=NMETA, c=CT)
                nc.sync.dma_start(Xv, src)
                outp = opool.tile([128, NT], BF16, tag="outp")
                tiles[t] = {"Xv": Xv, "outp": outp}

                if pe_reduce:
                    # product + one bf16 fold level; then PE reduces the
                    # remaining 64 d-slices into PSUM (f32 accumulate),
                    # yielding transposed scores [56=(m c), 128=n].
                    F1t = f1pool.tile([128, NMETA * CT * 64], BF16,
                                      tag="F1t")
                    F1v = F1t[:].rearrange("p (m c d) -> p m c d",
                                           m=NMETA, c=CT)
                    for g0, gl in GROUPS:
                        P = ppool.tile([128, NMETA * 4 * D], BF16, tag="P")
                        Pv = P[:].rearrange("p (m g d) -> p m g d",
                                            m=NMETA, g=4)
                        nc.vector.tensor_tensor(
                            out=Pv[:, :, :gl, :],
                            in0=Xv[:, :, g0:g0 + gl, :],
                            in1=wbv.unsqueeze(2).broadcast_to(
                                [128, NMETA, gl, D]),
                            op=ALU.mult,
                        )
                        with nc.allow_low_precision(reason="bf16 fold"):
                            nc.vector.tensor_tensor(
                                out=F1v[:, :, g0:g0 + gl, :],
                                in0=Pv[:, :, :gl, :64],
                                in1=Pv[:, :, :gl, 64:],
                                op=ALU.add,
                            )
                    ps_s = psum2.tile([128, 128], F32, tag="psS")
                    for d in range(64):
                        nc.tensor.matmul(
                            out=ps_s[:NMETA * CT, :],
                            lhsT=F1v[:, :, :, d],
                            rhs=icat[:, :D],
                            start=(d == 0),
                            stop=(d == 63),
                        )
                    # relu + exp on transposed scores
                    s_r = spool.tile([128, 128], F32, tag="s_r")
                    nc.scalar.activation(s_r[:NMETA * CT, :],
                                         ps_s[:NMETA * CT, :], ACTF.Relu)
                    E_T = spool.tile([128, 128], BF16, tag="E_T")
                    with nc.allow_low_precision(reason="bf16 e"):
                        nc.scalar.activation(E_T[:NMETA * CT, :],
                                             s_r[:NMETA * CT, :], ACTF.Exp)
                    # transpose back via perm -> [128 n, 56 (c m)] and copy
                    ps_e = psum2.tile([128, 64], F32, tag="psE")
                    nc.tensor.matmul(
                        out=ps_e[:, :NMETA * CT + CT],
                        lhsT=E_T[:NMETA * CT, :],
                        rhs=permt[:NMETA * CT, :NMETA * CT + CT],
                        start=True, stop=True,
                    )
                    e_bf = spool.tile([128, CT * NMETA], BF16, tag="e_bf")
                    with nc.allow_low_precision(reason="bf16 e"):
                        nc.scalar.activation(e_bf[:], ps_e[:, :NMETA * CT],
                                             ACTF.Copy)
                else:
                    # scores for the whole tile, (c, m)-ordered
                    scores_t = spool.tile([128, CT * NMETA], F32,
                                          tag="scores_t")
                    sv = scores_t[:].rearrange("p (c m) -> p m c", m=NMETA)
                    for g0, gl in GROUPS:
                        P = ppool.tile([128, NMETA * 4 * D], BF16, tag="P")
                        Pv = P[:].rearrange("p (m g d) -> p m g d",
                                            m=NMETA, g=4)
                        nc.vector.tensor_tensor(
                            out=Pv[:, :, :gl, :],
                            in0=Xv[:, :, g0:g0 + gl, :],
                            in1=wbv.unsqueeze(2).broadcast_to(
                                [128, NMETA, gl, D]),
                            op=ALU.mult,
                        )
                        w = D
                        cur = Pv
                        buf_iter = [f1pool, f2pool, f1pool, f2pool, f1pool,
                                    f2pool]
                        bi = 0
                        with nc.allow_low_precision(reason="bf16 fold tree"):
                            while w > fold_to:
                                h = w // 2
                                Fp = buf_iter[bi].tile(
                                    [128, NMETA * 4 * h], BF16, tag=f"F{h}")
                                bi += 1
                                Fv = Fp[:].rearrange(
                                    "p (m g d) -> p m g d", m=NMETA, g=4)
                                nc.vector.tensor_tensor(
                                    out=Fv[:, :, :gl, :],
                                    in0=cur[:, :, :gl, :h],
                                    in1=cur[:, :, :gl, h:w],
                                    op=ALU.add,
                                )
                                cur = Fv
                                w = h
                        nc.vector.tensor_reduce(
                            out=sv[:, :, g0:g0 + gl],
                            in_=cur[:, :, :gl, :w],
                            axis=mybir.AxisListType.X,
                            op=ALU.add,
                        )

                    # softmax tail: relu+exp on ACT, sums/recip/att on DVE
                    s_r = spool.tile([128, CT * NMETA], F32, tag="s_r")
                    nc.scalar.activation(s_r[:], scores_t[:], ACTF.Relu)
                    e_bf = spool.tile([128, CT * NMETA], BF16, tag="e_bf")
                    with nc.allow_low_precision(reason="bf16 softmax"):
                        nc.scalar.activation(e_bf[:], s_r[:], ACTF.Exp)
                inv = spool.tile([128, CT], F32, tag="inv")
                nc.vector.reciprocal(
                    inv[:], ps_e[:, NMETA * CT:NMETA * CT + CT])
                att_t = apool.tile([128, CT * NMETA], BF16, tag="att_t")
                av = att_t[:].rearrange("p (c m) -> p c m", m=NMETA)
                with nc.allow_low_precision(reason="bf16 att weights"):
                    nc.vector.tensor_tensor(
                        out=av[:],
                        in0=e_bf[:].rearrange("p (c m) -> p c m", m=NMETA),
                        in1=inv[:].unsqueeze(2).broadcast_to(
                            [128, CT, NMETA]),
                        op=ALU.mult,
                    )
                attf_t = apool.tile([128, CT * NMETA], F32, tag="attf_t")
                nc.scalar.activation(attf_t[:], att_t[:], ACTF.Copy)
                tiles[t]["att"] = att_t
                tiles[t]["attf"] = attf_t

            def emit_pool(t):
                Xv = tiles[t]["Xv"]
                att_t = tiles[t]["att"]
                attf_t = tiles[t]["attf"]
                pss = []
                for g0, gl in GROUPS:
                    ps = psum.tile([128, 4 * D], F32, tag="ps")
                    pss.append(ps)
                    for gg in range(gl):
                        c = g0 + gg
                        chunk_idx = t * CT + c
                        dg = dcpool.tile([128, NMETA * D], BF16,
                                         tag=f"dgc{c % 4}")
                        if diag_sched is not None:
                            eng = diag_sched[chunk_idx % len(diag_sched)]
                        else:
                            eng = ("A" if chunk_idx % diag_mod in act_pos
                                   else "P")
                        if eng == "P":
                            nc.gpsimd.local_scatter(
                                dg[:], att_t[:, c * NMETA:(c + 1) * NMETA],
                                sidx[:],
                                channels=128, num_elems=NMETA * D,
                                num_idxs=NMETA)
                        else:
                            # A: all 8 blocks on ACT; D: all on DVE;
                            # H: split ACT (m<4) / DVE (m>=4) to halve
                            # the serial build latency
                            for m in range(NMETA):
                                lowm = (m < hsplit)
                                if halt and chunk_idx % 2 == 1:
                                    lowm = not lowm
                                on_act = (eng == "A" or
                                          (eng == "H" and lowm))
                                if on_act:
                                    nc.scalar.activation(
                                        dg[:, m * D:(m + 1) * D],
                                        icat[:, m * D:(m + 1) * D],
                                        ACTF.Copy,
                                        scale=attf_t[:, c * NMETA + m:
                                                     c * NMETA + m + 1])
                                else:
                                    nc.vector.tensor_scalar(
                                        dg[:, m * D:(m + 1) * D],
                                        icat[:, m * D:(m + 1) * D],
                                        attf_t[:, c * NMETA + m:
                                               c * NMETA + m + 1],
                                        None, ALU.mult)
                        for m in range(NMETA):
                            nc.tensor.matmul(
                                out=ps[:, gg * D:(gg + 1) * D],
                                lhsT=dg[:, m * D:(m + 1) * D],
                                rhs=Xv[:, m, c, :],
                                start=(m == 0),
                                stop=(m == NMETA - 1),
                            )
                tiles[t]["pss"] = pss

            def emit_elu(t):
                outp = tiles[t]["outp"]
                pss = tiles[t]["pss"]
                for gi, (g0, gl) in enumerate(GROUPS):
                    ps = pss[gi]
                    # elu(x)+1 = relu(x) + exp(-relu(-x))  (store out+1)
                    w = gl * D
                    tneg = epool.tile([128, 4 * D], BF16, tag="tneg")
                    nc.scalar.activation(tneg[:, :w], ps[:, :w], ACTF.Relu,
                                         scale=-1.0)
                    e2 = epool.tile([128, 4 * D], BF16, tag="e2")
                    nc.scalar.activation(e2[:, :w], tneg[:, :w], ACTF.Exp,
                                         scale=-1.0)
                    r = epool.tile([128, 4 * D], BF16, tag="r")
                    nc.scalar.activation(r[:, :w], ps[:, :w], ACTF.Relu)
                    with nc.allow_low_precision(reason="bf16 out"):
                        nc.vector.tensor_tensor(
                            out=outp[:, g0 * D:g0 * D + w],
                            in0=r[:, :w], in1=e2[:, :w], op=ALU.add)
                n0 = t * NT
                dsto = out_d[n0:n0 + NT, :].rearrange(
                    "(p c) d -> p (c d)", p=128)
                nc.sync.dma_start(dsto, outp[:])
                del tiles[t]

            for t in range(NTILES):
                emit_front(t)
                if order == "fpe":
                    if t >= lag:
                        emit_pool(t - lag)
                    if t >= lag + 1:
                        emit_elu(t - lag - 1)
                else:
                    if t >= lag + 1:
                        emit_elu(t - lag - 1)
                    if t >= lag:
                        emit_pool(t - lag)
            for t in range(NTILES - lag, NTILES):
                emit_pool(t)
                emit_elu(t - 1)
            emit_elu(NTILES - 1)


def host_inputs(x_np, w_np, nc_pad=NC_PAD):
    """Build per-core input maps from full fp32 inputs."""
    in_maps = []
    wbig = np.ascontiguousarray(
        np.broadcast_to(w_np.T.reshape(1, NMETA * D), (128, NMETA * D))
    ).astype(ml_dtypes.bfloat16)
    sidx = (np.arange(NMETA)[None, :] * D
            + np.arange(128)[:, None]).astype(np.int16)
    # [56, 63]: cols 0-55 permute row (m*CT+c) -> col (c*NMETA+m);
    # cols 56-62 sum over m per chunk c
    perm = np.zeros((NMETA * CT, NMETA * CT + CT), dtype=np.float32)
    for m in range(NMETA):
        for c in range(CT):
            perm[m * CT + c, c * NMETA + m] = 1.0
            perm[m * CT + c, NMETA * CT + c] = 1.0
    perm = perm.astype(ml_dtypes.bfloat16)
    icat = np.ascontiguousarray(
        np.tile(np.eye(128, dtype=np.float32), (1, NMETA))
    ).astype(ml_dtypes.bfloat16)
    nc_raw = x_np.shape[1] // NCORES
    for c in range(NCORES):
        xs = x_np[:, c * nc_raw:(c + 1) * nc_raw, :]
        xp = np.zeros((NMETA, nc_pad, D), dtype=ml_dtypes.bfloat16)
        xp[:, :nc_raw, :] = xs.astype(ml_dtypes.bfloat16)
        in_maps.append({"x": xp, "wb": wbig, "sidx": sidx, "icat": icat,
                        "perm": perm})
    return in_maps


_CACHE = {}


def build(reps=1, **kw):
    key = (reps, tuple(sorted(kw.items())))
    if key in _CACHE:
        return _CACHE[key]
    nc = bacc.Bacc("TRN2", target_bir_lowering=False, debug=False,
                   num_devices=NCORES)
    x = nc.dram_tensor("x", [NMETA, NC_PAD, D], BF16, kind="ExternalInput").ap()
    wb = nc.dram_tensor("wb", [128, NMETA * D], BF16, kind="ExternalInput").ap()
    sidx = nc.dram_tensor("sidx", [128, NMETA], I16,
                          kind="ExternalInput").ap()
    icat = nc.dram_tensor("icat", [128, NMETA * D], BF16,
                          kind="ExternalInput").ap()
    perm = nc.dram_tensor("perm", [NMETA * CT, NMETA * CT + CT], BF16,
                          kind="ExternalInput").ap()
    out = nc.dram_tensor("out", [NC_PAD, D], BF16, kind="ExternalOutput").ap()
    with tile.TileContext(nc) as tc:
        kernel_body(tc, out, x, wb, sidx, icat, perm, reps=reps, **kw)
    nc.compile()
    _CACHE[key] = nc
    return nc


def run(input, W, trace=False, **trace_kwargs):
    x_np = np.asarray(input, dtype=np.float32)
    w_np = np.asarray(W, dtype=np.float32)
    nc = build()
    in_maps = host_inputs(x_np, w_np)
    res = bass_utils.run_bass_kernel_spmd(
        nc, in_maps, core_ids=list(range(NCORES)), trace=trace, **trace_kwargs)
    nc_raw = x_np.shape[1] // NCORES
    full = np.concatenate(
        [np.asarray(res.results[c]["out"][:nc_raw], dtype=np.float32) - 1.0
         for c in range(NCORES)], axis=0)
    return full, res


def kernel(input, W):
    out, _ = run(input, W, trace=False)
    return out
